# revision 7
# baseline (speedup 1.0000x reference)
"""Trainium2 Bass kernel for DiagonalMemoryOperator.

Computes out = x * (-|diag(W)|)  for x:[65536,2048] f32, W:[2048,2048] f32.

Strategy (data-parallel, per sharding hint): shard x rows across 8 cores
(8192 rows each); replicate the d-vector lam = diag(W) to every core; each
core streams its shard HBM->SBUF in big tiles, multiplies by the (device-
computed) -|lam| broadcast, and streams back.  Memory-bound: 64 MiB in +
64 MiB out per core.
"""

import numpy as np

import concourse.bass as bass
import concourse.tile as tile
from concourse import bacc, mybir
from concourse.alu_op_type import AluOpType
from concourse.bass_utils import run_bass_kernel_spmd

N, D = 65536, 2048
NCORES = 8
SHARD = N // NCORES  # 8192 rows per core
P = 128              # SBUF partitions
RPT = 4              # rows of x per partition per tile
F = RPT * D          # free elems per partition per tile (8192 -> 4 MiB tiles)
T = SHARD // (P * RPT)  # tiles per core (16)
WORK_BUFS = 4


def build(
    t=None,
    p=P,
    rpt=RPT,
    d=D,
    work_bufs=WORK_BUFS,
    ncores=NCORES,
    reps=1,
    variant="base",
):
    """Build + compile the per-core Bass module.

    DRAM views: x/out as [t, p, rpt*d] (a pure reshape of the row-contiguous
    [p*rpt*t, d] shard), lam replicated to [p, d] host-side.

    reps>1 unrolls the whole body multiple times inside one NEFF — used only
    for steady-state timing (marginal time per rep).

    variant: "base"  — loads on SP HWDGE ring, stores on ACT HWDGE ring
             "alt"   — ring assignment alternates with tile parity
             "swdge" — loads split SP/gpsimd, stores split ACT/gpsimd
             "empty" — no streaming body (NEFF-overhead calibration)
    """
    if t is None:
        assert SHARD % (p * rpt) == 0, (p, rpt)
        t = SHARD // (p * rpt)
    f = rpt * d
    nc = bacc.Bacc(
        "TRN2", target_bir_lowering=False, debug=False, num_devices=ncores
    )
    x = nc.dram_tensor("x", [t, p, f], mybir.dt.float32, kind="ExternalInput").ap()
    lam = nc.dram_tensor("lam", [p, d], mybir.dt.float32, kind="ExternalInput").ap()
    out = nc.dram_tensor("out", [t, p, f], mybir.dt.float32, kind="ExternalOutput").ap()

    with tile.TileContext(nc) as tc:
        with (
            tc.tile_pool(name="const", bufs=1) as cpool,
            tc.tile_pool(name="work", bufs=work_bufs) as wpool,
        ):
            lam_sb = cpool.tile([p, d], mybir.dt.float32)
            nc.sync.dma_start(lam_sb[:], lam[:])
            # lam_sb = -|lam| = min(lam * -1, lam)
            nc.vector.scalar_tensor_tensor(
                lam_sb[:], lam_sb[:], -1.0, lam_sb[:], AluOpType.mult, AluOpType.min
            )
            if variant == "empty":
                t = 0
            for _ in range(reps):
                for i in range(t):
                    if variant == "alt":
                        ld = nc.sync if i % 2 == 0 else nc.scalar
                        st = nc.scalar if i % 2 == 0 else nc.sync
                    elif variant == "swdge":
                        ld = nc.sync if i % 2 == 0 else nc.gpsimd
                        st = nc.scalar if i % 2 == 0 else nc.gpsimd
                    else:
                        # loads on SP's HWDGE ring, stores on ACT's, so load
                        # waits never head-of-line block behind compute waits
                        ld, st = nc.sync, nc.scalar
                    tl = wpool.tile([p, f], mybir.dt.float32)
                    ld.dma_start(tl[:], x[i])
                    for r in range(rpt):
                        sl = tl[:, r * d : (r + 1) * d]
                        nc.vector.tensor_mul(sl, sl, lam_sb[:])
                    st.dma_start(out[i], tl[:])
    nc.compile()
    return nc


_NC = None


def kernel(x: np.ndarray, W: np.ndarray) -> np.ndarray:
    global _NC
    if _NC is None:
        _NC = build()

    lam = np.ascontiguousarray(
        np.broadcast_to(np.diagonal(W), (P, D)), dtype=np.float32
    )
    in_maps = []
    for c in range(NCORES):
        xs = np.ascontiguousarray(x[c * SHARD : (c + 1) * SHARD]).reshape(T, P, F)
        in_maps.append({"x": xs, "lam": lam})

    res = run_bass_kernel_spmd(_NC, in_maps, list(range(NCORES)))
    outs = [res.results[c]["out"].reshape(SHARD, D) for c in range(NCORES)]
    return np.concatenate(outs, axis=0)
